# revision 1
# baseline (speedup 1.0000x reference)
"""Trainium2 Bass kernel v2 for nn_CategoricalDecoder (topk_masking).

Phase A (bin-sharded): tail-feature logits + local top-16 (3-term f32r
split matmuls). AllToAll flips to batch sharding. Phase B: merge, gather
winning z rows, exact fp32-class recompute of num/den on the 512 selected
columns, logsumexp.
"""

import numpy as np
from contextlib import ExitStack

import bass_rust as _br
import concourse.bass as bass
import concourse.bacc as bacc
import concourse.tile as tile
from concourse import mybir
from concourse.bass_utils import run_bass_kernel_spmd
from concourse.hw_specs import get_activation_tables

F32 = mybir.dt.float32
F32R = mybir.dt.float32r
U16 = mybir.dt.uint16
I16 = mybir.dt.int16
AF = mybir.ActivationFunctionType
ALU = mybir.AluOpType
AX = mybir.AxisListType

B, N, Lz, H, D, C = 256, 8192, 64, 256, 32, 16
DC = D * C
P = 8
NL = N // P
BL = B // P
K = 16
NEG = -1.0e30

# pk64 column offsets
O_ZTSH, O_ZTSL, O_W1H, O_W1L, O_OHT, O_B2T, O_G4 = (
    0, 1024, 2048, 2304, 2560, 2816, 2817)
PK64_COLS = 2821
# pk128 column offsets
O_W2H, O_W2L, O_B1, O_B2, O_OHB, O_GSEL, O_COEF, O_ONES, O_CO, O_IOTA, O_NCBT = (
    0, 1024, 2048, 2050, 2054, 2182, 2310, 2438, 2440, 2441, 2569)
PK128_COLS = 2570


class _Bacc(bacc.Bacc):
    """Bacc that pins every activation to the one table holding
    {Relu, Exp, Ln, Copy}, avoiding per-switch ACT_TABLE_LOADs."""

    def insert_act_table_loads(self):
        has_act = any(isinstance(i, mybir.InstActivation)
                      for b in self.main_func.blocks for i in b.instructions)
        if not has_act:
            return
        tables = []
        for name, funcs in get_activation_tables(self.m.arch).items():
            keep = funcs if name == "natural_log_exp_and_others" else set()
            tables.append((name, keep))
        _br.insert_act_table_loads(self, tables)


def _build_nc():
    nc = _Bacc("TRN2", target_bir_lowering=False, num_devices=P)

    dp = nc.declare_dram_parameter
    pk64 = dp("pk64", [Lz, PK64_COLS], F32R, isOutput=False)
    pk128 = dp("pk128", [128, PK128_COLS], F32R, isOutput=False)
    ztf = dp("ztf", [Lz, N], F32, isOutput=False)
    outp = dp("out", [BL], F32, isOutput=True)

    with tile.TileContext(nc) as tc, ExitStack() as ctx:
        const = ctx.enter_context(tc.tile_pool(name="const", bufs=1))
        dram = ctx.enter_context(tc.tile_pool(name="dram", bufs=1, space="DRAM"))

        k64 = const.tile([Lz, PK64_COLS], F32R, name="k64")
        nc.sync.dma_start(k64[:], pk64[:])
        k128 = const.tile([128, PK128_COLS], F32R, name="k128")
        nc.sync.dma_start(k128[:], pk128[:])
        ztf_sb = const.tile([Lz, N], F32, name="ztf_sb")
        nc.sync.dma_start(ztf_sb[:], ztf[:])

        def c64(off, w, p=Lz, dt=None):
            ap = k64[0:p, off:off + w]
            return ap.bitcast(dt) if dt else ap

        def c128(off, w, p=128, dt=None):
            ap = k128[0:p, off:off + w]
            return ap.bitcast(dt) if dt else ap

        xin = dram.tile([B, 16], F32)
        xout = dram.tile([B, 16], F32)

        # early dummy ap_gather: forces the gpsimd gather library load to
        # overlap the parameter DMAs instead of stalling phase B.
        with ExitStack() as ctx0:
            pre = ctx0.enter_context(tc.tile_pool(name="pre", bufs=1))
            zidx = pre.tile([16, 1], I16, name="zidx")
            nc.vector.memset(zidx[:], 0)
            junkg = pre.tile([16, 16], F32, name="junkg")
            nc.gpsimd.ap_gather(junkg[:], k64[0:16, 0:64].bitcast(F32), zidx[:],
                                channels=16, num_elems=64, d=1, num_idxs=16)

        # ================= phase A =================
        with ExitStack() as ctxA:
            pa = ctxA.enter_context(tc.tile_pool(name="pa", bufs=3, space="PSUM"))
            sp = ctxA.enter_context(tc.tile_pool(name="sp", bufs=1, space="PSUM"))
            act = ctxA.enter_context(tc.tile_pool(name="actA", bufs=1))
            scratch = ctxA.enter_context(tc.tile_pool(name="scrA", bufs=1))

            # hT = relu(W1.T @ zT + b1), 3-term f32r
            hs = []
            for m in range(2):
                ph = pa.tile([128, NL], F32, tag="mm")
                for f in range(2):
                    sl = slice(f * 512, (f + 1) * 512)
                    w1h = c64(O_W1H + m * 128, 128)
                    w1l = c64(O_W1L + m * 128, 128)
                    zh = c64(O_ZTSH + f * 512, 512)
                    zl = c64(O_ZTSL + f * 512, 512)
                    nc.tensor.matmul(ph[:, sl], w1h, zh, start=True, stop=False)
                    nc.tensor.matmul(ph[:, sl], w1h, zl, start=False, stop=False)
                    nc.tensor.matmul(ph[:, sl], w1l, zh, start=False, stop=True)
                b1 = c128(O_B1 + m, 1, dt=F32)
                hh = act.tile([128, NL], F32R, name=f"hh{m}")
                nc.scalar.activation(hh[:], ph[:], AF.Relu, bias=b1)
                hf = act.tile([128, NL], F32, name=f"hf{m}")
                nc.scalar.activation(hf[:], ph[:], AF.Relu, bias=b1)
                hl = act.tile([128, NL], F32R, name=f"hl{m}")
                nc.vector.tensor_sub(hl[:], hf[:], hh[:].bitcast(F32))
                hs.append((hh, hl))

            # tail logits (dc 448..512): [64, NL] (b2 folded out on host)
            pl3 = pa.tile([128, NL], F32, tag="mm")
            for f in range(2):
                sl = slice(f * 512, (f + 1) * 512)
                for kk in range(2):
                    w2h = c128(O_W2H + kk * DC + 448, 64)
                    w2l = c128(O_W2L + kk * DC + 448, 64)
                    hh, hl = hs[kk]
                    nc.tensor.matmul(pl3[0:64, sl], w2h, hh[:, sl],
                                     start=(kk == 0), stop=False)
                    nc.tensor.matmul(pl3[0:64, sl], w2h, hl[:, sl],
                                     start=False, stop=False)
                    nc.tensor.matmul(pl3[0:64, sl], w2l, hh[:, sl],
                                     start=False, stop=(kk == 1))
            b2t = c64(O_B2T, 1, dt=F32)
            e3r = act.tile([Lz, NL], F32R, name="e3r")
            nc.scalar.activation(e3r[:], pl3[0:64, :], AF.Exp, bias=b2t)
            l3h = act.tile([Lz, NL], F32R, name="l3h")
            nc.scalar.copy(l3h[:], pl3[0:64, :])
            l3l = act.tile([Lz, NL], F32R, name="l3l")
            nc.vector.tensor_sub(l3l[:], pl3[0:64, :], l3h[:].bitcast(F32))

            # log-sumexp of the 4 tail feature groups
            pse4 = sp.tile([4, NL], F32, tag="se")
            for f in range(2):
                sl = slice(f * 512, (f + 1) * 512)
                nc.tensor.matmul(pse4[:, sl], c64(O_G4, 4), e3r[:, sl],
                                 start=True, stop=True)
            l4h = act.tile([4, NL], F32R, name="l4h")
            nc.scalar.activation(l4h[:], pse4[:], AF.Ln)
            l4f = act.tile([4, NL], F32, name="l4f")
            nc.scalar.activation(l4f[:], pse4[:], AF.Ln)
            l4l = act.tile([4, NL], F32R, name="l4l")
            nc.vector.tensor_sub(l4l[:], l4f[:], l4h[:].bitcast(F32))

            # tail scores st[bt] [128, NL] = oht.T @ logits3 - sum(l4)
            for bt in range(2):
                pst = pa.tile([128, NL], F32, tag="mm")
                for f in range(2):
                    sl = slice(f * 512, (f + 1) * 512)
                    oht = c64(O_OHT + bt * 128, 128)
                    nc.tensor.matmul(pst[:, sl], oht, l3h[:, sl],
                                     start=True, stop=False)
                    nc.tensor.matmul(pst[:, sl], oht, l3l[:, sl],
                                     start=False, stop=False)
                    nc.tensor.matmul(pst[:, sl], c128(O_COEF, 128, p=4),
                                     l4h[:, sl], start=False, stop=False)
                    nc.tensor.matmul(pst[:, sl], c128(O_COEF, 128, p=4),
                                     l4l[:, sl], start=False, stop=True)

                # local top-8 + global ids, straight from PSUM
                x_sb = act.tile([128, 16], F32, name=f"x{bt}")
                nc.vector.max(x_sb[:, 0:8], pst[:])
                pu = act.tile([128, 8], U16, name=f"pu{bt}")
                nc.vector.max_index(pu[:], x_sb[:, 0:8], pst[:])
                nc.vector.tensor_copy(x_sb[:, 8:16], pu[:])
                nc.vector.tensor_scalar_add(x_sb[:, 8:16], x_sb[:, 8:16],
                                            c128(O_CO, 1, dt=F32))
                nc.sync.dma_start(xin[bt * 128:(bt + 1) * 128, :], x_sb[:])

        nc.gpsimd.collective_compute(
            "AllToAll", ALU.bypass, replica_groups=[list(range(P))],
            ins=[xin[:].opt()], outs=[xout[:].opt()],
        )

        # ================= phase B =================
        with ExitStack() as ctxB:
            pb = ctxB.enter_context(tc.tile_pool(name="pb", bufs=4, space="PSUM"))
            spb = ctxB.enter_context(tc.tile_pool(name="spb", bufs=1, space="PSUM"))
            act = ctxB.enter_context(tc.tile_pool(name="actB", bufs=1))
            scratch = ctxB.enter_context(tc.tile_pool(name="scrB", bufs=1))

            y = act.tile([BL, P, 16], F32, name="y")
            nc.sync.dma_start(y[:], xout[:].rearrange("(s p) f -> p s f", s=P))
            cands = act.tile([BL, P * 8], F32, name="cands")
            nc.vector.tensor_copy(
                cands[:].rearrange("p (a b) -> p a b", a=P), y[:, :, 0:8])
            idxc = act.tile([BL, P * 8], F32, name="idxc")
            nc.vector.tensor_copy(
                idxc[:].rearrange("p (a b) -> p a b", a=P), y[:, :, 8:16])

            wv = act.tile([BL, 16], F32, name="wv")
            nc.vector.max(wv[:, 0:8], cands[:])
            cm = act.tile([BL, P * 8], F32, name="cm")
            nc.vector.match_replace(cm[:], wv[:, 0:8], cands[:], NEG)
            nc.vector.max(wv[:, 8:16], cm[:])
            pw = act.tile([BL, 16], U16, name="pw")
            nc.vector.max_index(pw[:, 0:8], wv[:, 0:8], cands[:])
            nc.vector.max_index(pw[:, 8:16], wv[:, 8:16], cm[:])
            posf = act.tile([BL, 16], F32, name="posf")
            nc.vector.tensor_copy(posf[:], pw[:])

            widp = act.tile([32, 32], F32, name="widp")
            for j in range(16):
                junk = scratch.tile([BL, P * 8], F32, tag="junk")
                nc.vector.scalar_tensor_tensor(
                    junk[:], c128(O_IOTA, P * 8, p=BL, dt=F32), posf[:, j:j + 1],
                    idxc[:], op0=ALU.is_equal, op1=ALU.mult,
                    accum_out=widp[0:BL, j:j + 1])
            tp = act.tile([32, 32], F32, name="tp")
            nc.vector.transpose(tp[:], widp[:])
            idx64 = act.tile([Lz, 32], I16, name="idx64")
            nc.vector.tensor_copy(idx64[0:16, :], tp[0:16, :])
            for g in range(1, 4):
                nc.sync.dma_start(idx64[16 * g:16 * (g + 1), :], idx64[0:16, :])

            ztop = act.tile([Lz, 512], F32, name="ztop")
            nc.gpsimd.ap_gather(ztop[:], ztf_sb[:], idx64[:],
                                channels=Lz, num_elems=N, d=1, num_idxs=512)
            zh = act.tile([Lz, 512], F32R, name="zh")
            nc.vector.tensor_copy(zh[:], ztop[:])
            zl = act.tile([Lz, 512], F32R, name="zl")
            nc.vector.tensor_sub(zl[:], ztop[:], zh[:].bitcast(F32))

            h2s = []
            for m in range(2):
                ph2 = pb.tile([128, 512], F32, tag="mmb")
                w1h = c64(O_W1H + m * 128, 128)
                w1l = c64(O_W1L + m * 128, 128)
                nc.tensor.matmul(ph2[:], w1h, zh[:], start=True, stop=False)
                nc.tensor.matmul(ph2[:], w1h, zl[:], start=False, stop=False)
                nc.tensor.matmul(ph2[:], w1l, zh[:], start=False, stop=True)
                b1 = c128(O_B1 + m, 1, dt=F32)
                hh = act.tile([128, 512], F32R, name=f"hh2{m}")
                nc.scalar.activation(hh[:], ph2[:], AF.Relu, bias=b1)
                hf = act.tile([128, 512], F32, name=f"hf2{m}")
                nc.scalar.activation(hf[:], ph2[:], AF.Relu, bias=b1)
                hl = act.tile([128, 512], F32R, name=f"hl2{m}")
                nc.vector.tensor_sub(hl[:], hf[:], hh[:].bitcast(F32))
                h2s.append((hh, hl))

            pse2 = spb.tile([32, 512], F32, tag="seb")
            lin2s = []
            for t in range(4):
                pl2 = pb.tile([128, 512], F32, tag="mmb")
                for kk in range(2):
                    w2h = c128(O_W2H + kk * DC + t * 128, 128)
                    w2l = c128(O_W2L + kk * DC + t * 128, 128)
                    hh, hl = h2s[kk]
                    nc.tensor.matmul(pl2[:], w2h, hh[:], start=(kk == 0), stop=False)
                    nc.tensor.matmul(pl2[:], w2h, hl[:], start=False, stop=False)
                    nc.tensor.matmul(pl2[:], w2l, hh[:], start=False, stop=(kk == 1))
                b2 = c128(O_B2 + t, 1, dt=F32)
                e2r = act.tile([128, 512], F32R, name=f"e2r{t}")
                nc.scalar.activation(e2r[:], pl2[:], AF.Exp, bias=b2)
                lh = act.tile([128, 512], F32R, name=f"l2h{t}")
                nc.scalar.copy(lh[:], pl2[:])
                ll = act.tile([128, 512], F32R, name=f"l2l{t}")
                nc.vector.tensor_sub(ll[:], pl2[:], lh[:].bitcast(F32))
                lin2s.append((lh, ll))
                nc.tensor.matmul(pse2[:], c128(O_GSEL + t * 32, 32), e2r[:],
                                 start=(t == 0), stop=(t == 3))
            lgh = act.tile([32, 512], F32R, name="lgh")
            nc.scalar.activation(lgh[:], pse2[:], AF.Ln)
            lgf = act.tile([32, 512], F32, name="lgf")
            nc.scalar.activation(lgf[:], pse2[:], AF.Ln)
            lgl = act.tile([32, 512], F32R, name="lgl")
            nc.vector.tensor_sub(lgl[:], lgf[:], lgh[:].bitcast(F32))

            pnum = pb.tile([BL, 512], F32, tag="mmb")
            for t in range(4):
                lh, ll = lin2s[t]
                ohb = c128(O_OHB + t * BL, BL)
                nc.tensor.matmul(pnum[:], ohb, lh[:], start=(t == 0), stop=False)
                nc.tensor.matmul(pnum[:], ohb, ll[:], start=False, stop=False)
            nc.tensor.matmul(pnum[:], c128(O_COEF, BL, p=32), lgh[:],
                             start=False, stop=False)
            nc.tensor.matmul(pnum[:], c128(O_COEF, BL, p=32), lgl[:],
                             start=False, stop=True)
            numfull = act.tile([BL, 512], F32, name="numfull")
            nc.vector.tensor_copy(numfull[:], pnum[:])
            dscr = dram.tile([BL, 512], F32)
            nc.sync.dma_start(dscr[:], numfull[:])
            numd = act.tile([BL, 16], F32, name="numd")
            diag = bass.AP(tensor=dscr[:].tensor, offset=0,
                           ap=[[512 + 16, BL], [1, 16]])
            nc.sync.dma_start(numd[:], diag)

            # den = (numd + (-cbt)) - wv   (cbt: host-side tail-bias fold)
            den = act.tile([BL, 16], F32, name="den")
            nc.vector.scalar_tensor_tensor(
                den[:], numd[:], c128(O_NCBT, 1, p=BL, dt=F32), wv[:],
                op0=ALU.add, op1=ALU.subtract)
            ng = act.tile([BL, 2], F32, name="ng")
            nc.vector.tensor_reduce(ng[:, 0:1], numd[:], axis=AX.X, op=ALU.max,
                                    negate=True)
            nc.vector.tensor_reduce(ng[:, 1:2], den[:], axis=AX.X, op=ALU.max,
                                    negate=True)
            s2 = act.tile([BL, 2], F32, name="s2")
            en = scratch.tile([BL, 16], F32, tag="ex")
            nc.scalar.activation(en[:], numd[:], AF.Exp, bias=ng[:, 0:1],
                                 accum_out=s2[:, 0:1])
            ed = scratch.tile([BL, 16], F32, tag="ex")
            nc.scalar.activation(ed[:], den[:], AF.Exp, bias=ng[:, 1:2],
                                 accum_out=s2[:, 1:2])
            lg = act.tile([BL, 2], F32, name="lg")
            nc.scalar.activation(lg[:], s2[:], AF.Ln)
            t1 = act.tile([BL, 1], F32, name="t1")
            nc.vector.tensor_sub(t1[:], lg[:, 0:1], lg[:, 1:2])
            t2 = act.tile([BL, 1], F32, name="t2")
            nc.vector.tensor_sub(t2[:], ng[:, 1:2], ng[:, 0:1])
            t3 = act.tile([BL, 1], F32, name="t3")
            nc.vector.tensor_add(t3[:], t1[:], t2[:])
            nc.sync.dma_start(outp[:], t3[:, 0])

    nc.compile()
    return nc


def _trunc_split(a):
    a = np.ascontiguousarray(a, np.float32)
    hi = (a.view(np.uint32) & np.uint32(0xFFFFF000)).view(np.float32)
    lo = a - hi
    return hi, lo


def _host_prep(x, z, W1, b1, W2, b2):
    oh = np.zeros((B, DC), np.float32)
    oh[np.arange(B)[:, None], np.arange(D)[None, :] * C + x] = 1.0
    ohT = np.ascontiguousarray(oh.T)
    w2s = np.ascontiguousarray(
        W2.reshape(2, 128, DC).transpose(1, 0, 2).reshape(128, 2 * DC))
    w2h, w2l = _trunc_split(w2s)
    w1h, w1l = _trunc_split(W1)
    cbt = oh[:, 448:512] @ b2[448:512]          # (256,)

    k64c = np.zeros((Lz, PK64_COLS), np.float32)
    k64c[:, O_W1H:O_W1H + H] = w1h
    k64c[:, O_W1L:O_W1L + H] = w1l
    k64c[:, O_OHT:O_OHT + B] = ohT[448:512, :]
    k64c[:, O_B2T] = b2[448:512]
    g4 = np.zeros((Lz, 4), np.float32)
    g4[np.arange(Lz), np.arange(Lz) // 16] = 1.0
    k64c[:, O_G4:O_G4 + 4] = g4

    k128c = np.zeros((128, PK128_COLS), np.float32)
    k128c[:, O_W2H:O_W2H + 2 * DC] = w2h
    k128c[:, O_W2L:O_W2L + 2 * DC] = w2l
    k128c[:, O_B1:O_B1 + 2] = b1.reshape(2, 128).T
    k128c[:, O_B2:O_B2 + 4] = b2.reshape(4, 128).T
    p_idx = np.arange(128)
    for t in range(4):
        k128c[p_idx, O_GSEL + t * 32 + t * 8 + p_idx // 16] = 1.0
    k128c[0:32, O_COEF:O_COEF + 128] = -1.0
    k128c[:, O_ONES] = 1.0
    k128c[0:BL, O_IOTA:O_IOTA + 128] = np.arange(128, dtype=np.float32)[None, :]

    ztfull = np.ascontiguousarray(z.T)
    in_maps = []
    for c in range(P):
        kc64 = k64c.copy()
        zsh, zsl = _trunc_split(z[c * NL:(c + 1) * NL, :].T)
        kc64[:, O_ZTSH:O_ZTSH + NL] = zsh
        kc64[:, O_ZTSL:O_ZTSL + NL] = zsl
        kc128 = k128c.copy()
        kc128[:, O_CO] = c * NL
        for t in range(4):
            kc128[:, O_OHB + t * BL:O_OHB + (t + 1) * BL] = \
                ohT[t * 128:(t + 1) * 128, c * BL:(c + 1) * BL]
        kc128[0:BL, O_NCBT] = -cbt[c * BL:(c + 1) * BL]
        in_maps.append(dict(pk64=kc64, pk128=kc128, ztf=ztfull))
    return in_maps


_NC_CACHE = {}


def kernel(x, log_w, z, k, W1, b1, W2, b2, _trace=False, _trace_kwargs=None):
    assert int(k) == K
    in_maps = _host_prep(np.asarray(x, np.int32), np.asarray(z, np.float32),
                         np.asarray(W1, np.float32), np.asarray(b1, np.float32),
                         np.asarray(W2, np.float32), np.asarray(b2, np.float32))
    if "nc" not in _NC_CACHE:
        _NC_CACHE["nc"] = _build_nc()
    nc = _NC_CACHE["nc"]
    res = run_bass_kernel_spmd(
        nc, in_maps, list(range(P)), trace=_trace, **(_trace_kwargs or {}))
    if _trace:
        _NC_CACHE["last_result"] = res
    return np.concatenate([np.asarray(res.results[c]["out"], np.float32)
                           for c in range(P)])



# revision 17
# speedup vs baseline: 1.4981x; 1.4981x over previous
"""Trainium2 Bass kernel v3 for nn_CategoricalDecoder (topk_masking).

Phase A (bin-sharded, single-term f32r): tail-feature logits for the local
1024-bin shard, scores packed as int32 (score<<13 | global bin id), local
top-8 per batch row via max8. AllToAll flips to batch sharding (8KB).
Phase B: merge 64 candidates/row -> top-16 packed (ids come free via
bitwise AND), dma_gather of winner z rows from DRAM, single-term f32r
recompute of num/den (den exact via host-folded oh@W2 matmuls), logsumexp.
"""

import numpy as np
from contextlib import ExitStack

import bass_rust as _br
import concourse.bass as bass
import concourse.bacc as bacc
import concourse.tile as tile
from concourse import mybir
from concourse.bass_utils import run_bass_kernel_spmd
from concourse.hw_specs import get_activation_tables

F32 = mybir.dt.float32
F32R = mybir.dt.float32r
I32 = mybir.dt.int32
I16 = mybir.dt.int16
AF = mybir.ActivationFunctionType
ALU = mybir.AluOpType
AX = mybir.AxisListType

B, N, Lz, H, D, C = 256, 8192, 64, 256, 32, 16
DC = D * C
P = 8
NL = N // P
BL = B // P
K = 16
NTAIL = 64  # tail-feature logit rows (4 features x 16 classes)

# pk64 column offsets ([64, C64])
O_ZT, O_W1, O_OHT, O_G4, O_B2T = 0, 1024, 1280, 1536, 1540
C64 = 1541
# pk128a column offsets ([128, C128A]) -- small, loaded early
A_W2T, A_COEF, A_B1, A_BASE = 0, 128, 256, 258
C128A = 259
# pk128b column offsets ([128, C128B]) -- phase B constants
B_W2, B_GSEL, B_WOHND, B_COEFND, B_B2E, B_CBND, B_IDENT = (
    0, 1024, 1152, 1280, 1344, 1348, 1350)
C128B = 1478


def _stt_int(eng, out, in0, imm, in1, op0, op1):
    """scalar_tensor_tensor with an int32-typed immediate (bitvec ops)."""
    return eng.add_instruction(
        mybir.InstTensorScalarPtr(
            name=eng.bass.get_next_instruction_name(),
            is_scalar_tensor_tensor=True,
            op0=op0, op1=op1,
            ins=[eng.lower_ap(in0),
                 mybir.ImmediateValue(dtype=I32, value=imm),
                 eng.lower_ap(in1)],
            outs=[eng.lower_ap(out)]))


def _ts_int(eng, out, in0, imm, op0):
    """tensor_scalar with an int32-typed immediate (bitvec ops)."""
    return eng.add_instruction(
        mybir.InstTensorScalarPtr(
            name=eng.bass.get_next_instruction_name(),
            op0=op0,
            ins=[eng.lower_ap(in0),
                 mybir.ImmediateValue(dtype=I32, value=imm)],
            outs=[eng.lower_ap(out)]))


class _Bacc(bacc.Bacc):
    """Bacc that pins every activation to the one table holding
    {Relu, Exp, Ln, Copy}, avoiding per-switch ACT_TABLE_LOADs."""

    def insert_act_table_loads(self):
        has_act = any(isinstance(i, mybir.InstActivation)
                      for b in self.main_func.blocks for i in b.instructions)
        if not has_act:
            return
        tables = []
        for name, funcs in get_activation_tables(self.m.arch).items():
            keep = funcs if name == "natural_log_exp_and_others" else set()
            tables.append((name, keep))
        _br.insert_act_table_loads(self, tables)


def _build_nc():
    nc = _Bacc("TRN2", target_bir_lowering=False, num_devices=P)

    dp = nc.declare_dram_parameter
    pk64 = dp("pk64", [Lz, C64], F32R, isOutput=False)
    pk128a = dp("pk128a", [128, C128A], F32R, isOutput=False)
    pk128b = dp("pk128b", [128, C128B], F32R, isOutput=False)
    zr = dp("zr", [N, Lz], F32, isOutput=False)
    outp = dp("out", [BL], F32, isOutput=True)
    dbgi = dp("dbgids", [16, 32], I16, isOutput=True)

    with tile.TileContext(nc) as tc, ExitStack() as ctx:
        const = ctx.enter_context(tc.tile_pool(name="const", bufs=1))
        dram = ctx.enter_context(tc.tile_pool(name="dram", bufs=1, space="DRAM"))

        k64 = const.tile([Lz, C64], F32R, name="k64")
        nc.sync.dma_start(k64[:], pk64[:])
        ka = const.tile([128, C128A], F32R, name="ka")
        nc.sync.dma_start(ka[:], pk128a[:])
        kb = const.tile([128, C128B], F32R, name="kb")
        nc.sync.dma_start(kb[:], pk128b[:])

        def c64(off, w, p=Lz, dt=None):
            ap = k64[0:p, off:off + w]
            return ap.bitcast(dt) if dt else ap

        def ca(off, w, p=128, dt=None):
            ap = ka[0:p, off:off + w]
            return ap.bitcast(dt) if dt else ap

        def cb(off, w, p=128, dt=None):
            ap = kb[0:p, off:off + w]
            return ap.bitcast(dt) if dt else ap

        xin = dram.tile([B, 8], F32)
        xout = dram.tile([B, 8], F32)
        dscr = dram.tile([2 * BL, 512], F32)

        # local bin ids 0..1023 (shard recovered in phase B from slot pos)
        lid = const.tile([128, NL], I32, name="lid")
        nc.gpsimd.iota(lid[:], pattern=[[1, NL]], base=0, channel_multiplier=0)

        # dummy dma_gather: forces the SWDGE library load to overlap the
        # parameter DMAs instead of stalling phase B.
        with ExitStack() as ctx0:
            pre = ctx0.enter_context(tc.tile_pool(name="pre", bufs=1))
            idxd = pre.tile([128, 8], I16, name="idxd")
            nc.vector.memset(idxd[:], 0)
            outd = pre.tile([128, 1, Lz], F32, name="outd")
            nc.gpsimd.dma_gather(outd[:], zr[:], idxd[:], num_idxs=128,
                                 num_idxs_reg=128, elem_size=Lz)

        # ================= phase A =================
        with ExitStack() as ctxA:
            pa = ctxA.enter_context(tc.tile_pool(name="pa", bufs=3, space="PSUM"))
            sp = ctxA.enter_context(tc.tile_pool(name="sp", bufs=1, space="PSUM"))
            act = ctxA.enter_context(tc.tile_pool(name="actA", bufs=1))
            scr = ctxA.enter_context(tc.tile_pool(name="scrA", bufs=2))

            # h = relu(W1.T @ zT + b1): [256, NL] as 2 m-tiles (full f32:
            # the score path must match the reference top-16 exactly)
            hs = []
            for m in range(2):
                ph = pa.tile([128, NL], F32, tag="mm")
                for f in range(2):
                    sl = slice(f * 512, (f + 1) * 512)
                    nc.tensor.matmul(ph[:, sl],
                                     c64(O_W1 + m * 128, 128, dt=F32),
                                     c64(O_ZT + f * 512, 512, dt=F32),
                                     start=True, stop=True)
                hh = act.tile([128, NL], F32, name=f"hh{m}")
                nc.scalar.activation(hh[:], ph[:], AF.Relu,
                                     bias=ca(A_B1 + m, 1, dt=F32))
                hs.append(hh)

            # tail logits l3 [64, NL] (no b2: constant-in-n for ranking)
            pl3 = pa.tile([128, NL], F32, tag="mm")
            for f in range(2):
                sl = slice(f * 512, (f + 1) * 512)
                for kk in range(2):
                    nc.tensor.matmul(pl3[0:NTAIL, sl],
                                     ca(A_W2T + kk * NTAIL, NTAIL, dt=F32),
                                     hs[kk][:, sl],
                                     start=(kk == 0), stop=(kk == 1))
            l3r = act.tile([NTAIL, NL], F32, name="l3r")
            nc.scalar.copy(l3r[:], pl3[0:NTAIL, :])
            e3r = act.tile([NTAIL, NL], F32, name="e3r")
            nc.scalar.activation(e3r[:], pl3[0:NTAIL, :], AF.Exp,
                                 bias=c64(O_B2T, 1, dt=F32))

            # log-sum-exp of the 4 tail feature groups
            pse4 = sp.tile([4, NL], F32, tag="se")
            for f in range(2):
                sl = slice(f * 512, (f + 1) * 512)
                nc.tensor.matmul(pse4[:, sl], c64(O_G4, 4, dt=F32),
                                 e3r[:, sl], start=True, stop=True)
            l4r = act.tile([4, NL], F32, name="l4r")
            nc.scalar.activation(l4r[:], pse4[:], AF.Ln)

            # scores scaled by 2^14 (folded into oht/coef on host), packed
            # as int32: clamp0(16384*s + 2^19) << 10 | local_id, top-8
            for bt in range(2):
                pst = pa.tile([128, NL], F32, tag="mm")
                for f in range(2):
                    sl = slice(f * 512, (f + 1) * 512)
                    nc.tensor.matmul(pst[:, sl],
                                     c64(O_OHT + bt * 128, 128, dt=F32),
                                     l3r[:, sl], start=True, stop=False)
                    nc.tensor.matmul(pst[:, sl], ca(A_COEF, 128, p=4, dt=F32),
                                     l4r[:, sl], start=False, stop=True)
                t32 = scr.tile([128, NL], I32, tag="t32")
                nc.vector.tensor_scalar(t32[:], pst[:], 524288.0, 0.0,
                                        op0=ALU.add, op1=ALU.max)
                pk = scr.tile([128, NL], I32, tag="pk")
                _stt_int(nc.vector, pk[:], t32[:], 10, lid[:],
                         ALU.logical_shift_left, ALU.bitwise_or)
                x_sb = act.tile([128, 8], F32, name=f"x{bt}")
                nc.vector.max(x_sb[:], pk[:].bitcast(F32))
                nc.sync.dma_start(xin[bt * 128:(bt + 1) * 128, :], x_sb[:])

        nc.gpsimd.collective_compute(
            "AllToAll", ALU.bypass, replica_groups=[list(range(P))],
            ins=[xin[:].opt()], outs=[xout[:].opt()],
        )

        # ================= phase B =================
        with ExitStack() as ctxB:
            pb = ctxB.enter_context(tc.tile_pool(name="pb", bufs=4, space="PSUM"))
            spb = ctxB.enter_context(tc.tile_pool(name="spb", bufs=1, space="PSUM"))
            ptr = ctxB.enter_context(tc.tile_pool(name="ptr", bufs=1, space="PSUM"))
            pnd_p = ctxB.enter_context(tc.tile_pool(name="pnd", bufs=1, space="PSUM"))
            act = ctxB.enter_context(tc.tile_pool(name="actB", bufs=1))

            # 64 packed candidates per local batch row
            cands = act.tile([BL, P * 8], F32, name="cands")
            nc.sync.dma_start(cands[:],
                              xout[:].rearrange("(s p) f -> p s f", s=P))
            wv = act.tile([BL, 16], F32, name="wv")
            nc.vector.max(wv[:, 0:8], cands[:])
            cm = act.tile([BL, P * 8], F32, name="cm")
            nc.vector.match_replace(cm[:], wv[:, 0:8], cands[:], 0.0)
            nc.vector.max(wv[:, 8:16], cm[:])

            # winner global ids: shard from slot position (pos//8)*1024,
            # local id from the packed low 10 bits
            pu = act.tile([BL, 16], mybir.dt.uint16, name="pu")
            nc.vector.max_index(pu[:, 0:8], wv[:, 0:8], cands[:])
            nc.vector.max_index(pu[:, 8:16], wv[:, 8:16], cands[:])
            posI = act.tile([BL, 16], I32, name="posI")
            nc.vector.tensor_copy(posI[:], pu[:])
            m1 = act.tile([BL, 16], I32, name="m1")
            _ts_int(nc.vector, m1[:], posI[:], 7, ALU.logical_shift_left)
            m2 = act.tile([BL, 16], I32, name="m2")
            _ts_int(nc.vector, m2[:], m1[:], 0x1C00, ALU.bitwise_and)
            ids32 = act.tile([32, 32], I32, name="ids32")
            nc.vector.memset(ids32[:], 0)
            _stt_int(nc.vector, ids32[:, 0:16], wv[:].bitcast(I32), 1023,
                     m2[:], ALU.bitwise_and, ALU.bitwise_or)
            idT = act.tile([32, 32], I32, name="idT")
            nc.vector.transpose(idT[:], ids32[:])
            idxall = act.tile([128, 32], I16, name="idxall")
            nc.vector.tensor_copy(idxall[0:16, :], idT[0:16, :])
            nc.sync.dma_start(dbgi[:], idxall[0:16, :])
            for g in range(1, 8):
                nc.sync.dma_start(idxall[16 * g:16 * (g + 1), :],
                                  idxall[0:16, :])

            # gather winner z rows from DRAM: ztop[p, i, :] = z[idx[i*128+p]]
            ztop = act.tile([128, 4, Lz], F32, name="ztop")
            nc.gpsimd.dma_gather(ztop[:], zr[:], idxall[:], num_idxs=512,
                                 num_idxs_reg=512, elem_size=Lz)
            # transpose to [64, 512] via PE
            pzt = ptr.tile([Lz, 512], F32, tag="tr")
            for i in range(4):
                nc.tensor.transpose(pzt[:, i * 128:(i + 1) * 128],
                                    ztop[:, i, :].bitcast(F32),
                                    cb(B_IDENT, 128, dt=F32))
            ztr = act.tile([Lz, 512], F32R, name="ztr")
            nc.scalar.copy(ztr[:], pzt[:])

            # h2 = relu(W1.T @ ztop + b1)
            h2s = []
            for m in range(2):
                ph2 = pb.tile([128, 512], F32, tag="mmb")
                nc.tensor.matmul(ph2[:], c64(O_W1 + m * 128, 128), ztr[:],
                                 start=True, stop=True)
                hh2 = act.tile([128, 512], F32R, name=f"hh2{m}")
                nc.scalar.activation(hh2[:], ph2[:], AF.Relu,
                                     bias=ca(A_B1 + m, 1, dt=F32))
                h2s.append(hh2)

            # full logits per t-tile -> exp -> per-feature sums
            pse2 = spb.tile([32, 512], F32, tag="seb")
            for t in range(4):
                pl2 = pb.tile([128, 512], F32, tag="mmb")
                for kk in range(2):
                    nc.tensor.matmul(pl2[:], cb(B_W2 + (kk * 4 + t) * 128, 128),
                                     h2s[kk][:], start=(kk == 0), stop=(kk == 1))
                e2r = act.tile([128, 512], F32R, name=f"e2r{t}")
                nc.scalar.activation(e2r[:], pl2[:], AF.Exp,
                                     bias=cb(B_B2E + t, 1, dt=F32))
                nc.tensor.matmul(pse2[:], cb(B_GSEL + t * 32, 32), e2r[:],
                                 start=(t == 0), stop=(t == 3))
            lgr = act.tile([32, 512], F32R, name="lgr")
            nc.scalar.activation(lgr[:], pse2[:], AF.Ln)

            # num (rows 0-31) and den (rows 32-63) in one PSUM tile
            pnd = pnd_p.tile([2 * BL, 512], F32, tag="nd")
            for kk in range(2):
                nc.tensor.matmul(pnd[:], cb(B_WOHND + kk * 64, 64), h2s[kk][:],
                                 start=(kk == 0), stop=False)
            nc.tensor.matmul(pnd[:], cb(B_COEFND, 64, p=32), lgr[:],
                             start=False, stop=True)
            ndf = act.tile([2 * BL, 512], F32, name="ndf")
            nc.scalar.copy(ndf[:], pnd[:])
            nc.sync.dma_start(dscr[:], ndf[:])

            # diagonal extraction: numd[b, j] = ndf[b, b*16+j] (+ den half)
            numd = act.tile([BL, 32], F32, name="numd")
            dnum = bass.AP(tensor=dscr[:].tensor, offset=0,
                           ap=[[512 + 16, BL], [1, 16]])
            nc.sync.dma_start(numd[:, 0:16], dnum)
            dden = bass.AP(tensor=dscr[:].tensor, offset=BL * 512,
                           ap=[[512 + 16, BL], [1, 16]])
            nc.sync.dma_start(numd[:, 16:32], dden)
            # fold oh.b2 constants (zero for this model, kept for generality)
            nc.vector.tensor_scalar_add(numd[:, 0:16], numd[:, 0:16],
                                        cb(B_CBND, 1, p=BL, dt=F32))
            nc.vector.tensor_scalar_add(numd[:, 16:32], numd[:, 16:32],
                                        cb(B_CBND + 1, 1, p=BL, dt=F32))

            # logsumexp(num) - logsumexp(den)
            ng = act.tile([BL, 2], F32, name="ng")
            nc.vector.tensor_reduce(ng[:, 0:1], numd[:, 0:16], axis=AX.X,
                                    op=ALU.max, negate=True)
            nc.vector.tensor_reduce(ng[:, 1:2], numd[:, 16:32], axis=AX.X,
                                    op=ALU.max, negate=True)
            s2 = act.tile([BL, 2], F32, name="s2")
            en = act.tile([BL, 16], F32, name="en")
            nc.scalar.activation(en[:], numd[:, 0:16], AF.Exp,
                                 bias=ng[:, 0:1], accum_out=s2[:, 0:1])
            ed = act.tile([BL, 16], F32, name="ed")
            nc.scalar.activation(ed[:], numd[:, 16:32], AF.Exp,
                                 bias=ng[:, 1:2], accum_out=s2[:, 1:2])
            lg = act.tile([BL, 2], F32, name="lg")
            nc.scalar.activation(lg[:], s2[:], AF.Ln)
            t1 = act.tile([BL, 1], F32, name="t1")
            nc.vector.tensor_sub(t1[:], lg[:, 0:1], lg[:, 1:2])
            t2 = act.tile([BL, 1], F32, name="t2")
            nc.vector.tensor_sub(t2[:], ng[:, 1:2], ng[:, 0:1])
            t3 = act.tile([BL, 1], F32, name="t3")
            nc.vector.tensor_add(t3[:], t1[:], t2[:])
            nc.sync.dma_start(outp[:], t3[:, 0])

    nc.compile()
    return nc


def _host_prep(x, z, W1, b1, W2, b2):
    oh = np.zeros((B, DC), np.float32)
    oh[np.arange(B)[:, None], np.arange(D)[None, :] * C + x] = 1.0
    oh_obs = oh.copy()
    oh_obs[:, DC - 4 * C:] = 0.0
    woh = oh @ W2.T          # (256, 256)
    wohd = oh_obs @ W2.T
    cbn = oh @ b2            # (256,)
    cbd = oh_obs @ b2

    k64c = np.zeros((Lz, C64), np.float32)
    k64c[:, O_W1:O_W1 + H] = W1
    # tail one-hot transposed, scaled by 2^14 for score packing
    k64c[:, O_OHT:O_OHT + B] = 16384.0 * oh[:, DC - NTAIL:].T
    g4 = np.zeros((Lz, 4), np.float32)
    g4[np.arange(Lz), np.arange(Lz) // C] = 1.0
    k64c[:, O_G4:O_G4 + 4] = g4
    k64c[:, O_B2T] = b2[DC - NTAIL:]

    kac = np.zeros((128, C128A), np.float32)
    for kk in range(2):
        kac[:, A_W2T + kk * NTAIL:A_W2T + (kk + 1) * NTAIL] = \
            W2[kk * 128:(kk + 1) * 128, DC - NTAIL:]
    kac[0:4, A_COEF:A_COEF + 128] = -16384.0
    kac[:, A_B1:A_B1 + 2] = b1.reshape(2, 128).T

    kbc = np.zeros((128, C128B), np.float32)
    for kk in range(2):
        for t in range(4):
            kbc[:, B_W2 + (kk * 4 + t) * 128:B_W2 + (kk * 4 + t + 1) * 128] = \
                W2[kk * 128:(kk + 1) * 128, t * 128:(t + 1) * 128]
    p_idx = np.arange(128)
    for t in range(4):
        kbc[p_idx, B_GSEL + t * 32 + t * 8 + p_idx // C] = 1.0
    kbc[0:32, B_COEFND:B_COEFND + 32] = -1.0
    kbc[0:28, B_COEFND + 32:B_COEFND + 64] = -1.0
    kbc[:, B_B2E:B_B2E + 4] = b2.reshape(4, 128).T
    kbc[:, B_IDENT:B_IDENT + 128] = np.eye(128, dtype=np.float32)

    zf = np.ascontiguousarray(z, np.float32)
    in_maps = []
    for c in range(P):
        kc64 = k64c.copy()
        kc64[:, O_ZT:O_ZT + NL] = z[c * NL:(c + 1) * NL, :].T
        kca = kac.copy()
        kcb = kbc.copy()
        bsl = slice(c * BL, (c + 1) * BL)
        for kk in range(2):
            kcb[:, B_WOHND + kk * 64:B_WOHND + kk * 64 + BL] = \
                woh[bsl, kk * 128:(kk + 1) * 128].T
            kcb[:, B_WOHND + kk * 64 + BL:B_WOHND + (kk + 1) * 64] = \
                wohd[bsl, kk * 128:(kk + 1) * 128].T
        kcb[0:BL, B_CBND] = cbn[bsl]
        kcb[0:BL, B_CBND + 1] = cbd[bsl]
        in_maps.append(dict(pk64=kc64, pk128a=kca, pk128b=kcb, zr=zf))
    return in_maps


_NC_CACHE = {}


def kernel(x, log_w, z, k, W1, b1, W2, b2, _trace=False, _trace_kwargs=None):
    assert int(k) == K
    in_maps = _host_prep(np.asarray(x, np.int32), np.asarray(z, np.float32),
                         np.asarray(W1, np.float32), np.asarray(b1, np.float32),
                         np.asarray(W2, np.float32), np.asarray(b2, np.float32))
    if "nc" not in _NC_CACHE:
        _NC_CACHE["nc"] = _build_nc()
    nc = _NC_CACHE["nc"]
    res = run_bass_kernel_spmd(
        nc, in_maps, list(range(P)), trace=_trace, **(_trace_kwargs or {}))
    if _trace:
        _NC_CACHE["last_result"] = res
    return np.concatenate([np.asarray(res.results[c]["out"], np.float32)
                           for c in range(P)])


# revision 25
# speedup vs baseline: 1.6673x; 1.1130x over previous
"""Trainium2 Bass kernel v3 for nn_CategoricalDecoder (topk_masking).

Phase A (bin-sharded, single-term f32r): tail-feature logits for the local
1024-bin shard, scores packed as int32 (score<<13 | global bin id), local
top-8 per batch row via max8. AllToAll flips to batch sharding (8KB).
Phase B: merge 64 candidates/row -> top-16 packed (ids come free via
bitwise AND), dma_gather of winner z rows from DRAM, single-term f32r
recompute of num/den (den exact via host-folded oh@W2 matmuls), logsumexp.
"""

import numpy as np
from contextlib import ExitStack

import bass_rust as _br
import concourse.bass as bass
import concourse.bacc as bacc
import concourse.tile as tile
from concourse import mybir
from concourse.bass_utils import run_bass_kernel_spmd
from concourse.hw_specs import get_activation_tables

F32 = mybir.dt.float32
F32R = mybir.dt.float32r
I32 = mybir.dt.int32
I16 = mybir.dt.int16
AF = mybir.ActivationFunctionType
ALU = mybir.AluOpType
AX = mybir.AxisListType

B, N, Lz, H, D, C = 256, 8192, 64, 256, 32, 16
DC = D * C
P = 8
NL = N // P
BL = B // P
K = 16
NTAIL = 64  # tail-feature logit rows (4 features x 16 classes)

# pk64 column offsets ([64, C64])
O_ZT, O_W1, O_OHT, O_G4, O_B2T = 0, 1024, 1280, 1536, 1540
C64 = 1541
# pk128a column offsets ([128, C128A]) -- small, loaded early
A_W2T, A_COEF, A_B1, A_BASE = 0, 128, 256, 258
C128A = 259
# pk128b column offsets ([128, C128B]) -- phase B constants
B_W2, B_GSEL, B_WOHND, B_COEFND, B_B2E, B_CBND, B_IDENT = (
    0, 1024, 1152, 1280, 1344, 1348, 1350)
B_MASK = 1478
C128B = 1494


def _stt_int(eng, out, in0, imm, in1, op0, op1):
    """scalar_tensor_tensor with an int32-typed immediate (bitvec ops)."""
    return eng.add_instruction(
        mybir.InstTensorScalarPtr(
            name=eng.bass.get_next_instruction_name(),
            is_scalar_tensor_tensor=True,
            op0=op0, op1=op1,
            ins=[eng.lower_ap(in0),
                 mybir.ImmediateValue(dtype=I32, value=imm),
                 eng.lower_ap(in1)],
            outs=[eng.lower_ap(out)]))


def _ts_int(eng, out, in0, imm, op0):
    """tensor_scalar with an int32-typed immediate (bitvec ops)."""
    return eng.add_instruction(
        mybir.InstTensorScalarPtr(
            name=eng.bass.get_next_instruction_name(),
            op0=op0,
            ins=[eng.lower_ap(in0),
                 mybir.ImmediateValue(dtype=I32, value=imm)],
            outs=[eng.lower_ap(out)]))


class _Bacc(bacc.Bacc):
    """Bacc that pins every activation to the one table holding
    {Relu, Exp, Ln, Copy}, avoiding per-switch ACT_TABLE_LOADs."""

    def insert_act_table_loads(self):
        has_act = any(isinstance(i, mybir.InstActivation)
                      for b in self.main_func.blocks for i in b.instructions)
        if not has_act:
            return
        tables = []
        for name, funcs in get_activation_tables(self.m.arch).items():
            keep = funcs if name == "natural_log_exp_and_others" else set()
            tables.append((name, keep))
        _br.insert_act_table_loads(self, tables)


def _build_nc():
    nc = _Bacc("TRN2", target_bir_lowering=False, num_devices=P)

    dp = nc.declare_dram_parameter
    pk64 = dp("pk64", [Lz, C64], F32R, isOutput=False)
    pk128a = dp("pk128a", [128, C128A], F32R, isOutput=False)
    pk128b = dp("pk128b", [128, C128B], F32R, isOutput=False)
    zr = dp("zr", [N, Lz], F32, isOutput=False)
    outp = dp("out", [BL], F32, isOutput=True)
    dbgi = dp("dbgids", [16, 32], I16, isOutput=True)

    with tile.TileContext(nc) as tc, ExitStack() as ctx:
        const = ctx.enter_context(tc.tile_pool(name="const", bufs=1))
        dram = ctx.enter_context(tc.tile_pool(name="dram", bufs=1, space="DRAM"))

        k64 = const.tile([Lz, C64], F32R, name="k64")
        nc.sync.dma_start(k64[:], pk64[:])
        ka = const.tile([128, C128A], F32R, name="ka")
        nc.sync.dma_start(ka[:], pk128a[:])
        kb = const.tile([128, C128B], F32R, name="kb")
        nc.sync.dma_start(kb[:], pk128b[:])

        def c64(off, w, p=Lz, dt=None):
            ap = k64[0:p, off:off + w]
            return ap.bitcast(dt) if dt else ap

        def ca(off, w, p=128, dt=None):
            ap = ka[0:p, off:off + w]
            return ap.bitcast(dt) if dt else ap

        def cb(off, w, p=128, dt=None):
            ap = kb[0:p, off:off + w]
            return ap.bitcast(dt) if dt else ap

        xin = dram.tile([B, 8], F32)
        xout = dram.tile([B, 8], F32)
        dscr = dram.tile([2 * BL, 512], F32)

        # local bin ids 0..1023 (shard recovered in phase B from slot pos)
        lid = const.tile([128, NL], I32, name="lid")
        nc.gpsimd.iota(lid[:], pattern=[[1, NL]], base=0, channel_multiplier=0)

        # dummy dma_gather + dummy activation: force the SWDGE library load
        # and the ACT table load to overlap the parameter DMAs instead of
        # stalling phase A/B.
        with ExitStack() as ctx0:
            pre = ctx0.enter_context(tc.tile_pool(name="pre", bufs=1))
            idxd = pre.tile([128, 8], I16, name="idxd")
            nc.vector.memset(idxd[:], 0)
            outd = pre.tile([128, 1, Lz], F32, name="outd")
            nc.gpsimd.dma_gather(outd[:], zr[:], idxd[:], num_idxs=128,
                                 num_idxs_reg=128, elem_size=Lz)
            ja = pre.tile([1, 2], F32, name="ja")
            nc.vector.memset(ja[:], 0)
            jb = pre.tile([1, 2], F32, name="jb")
            nc.scalar.activation(jb[:], ja[:], AF.Relu)

        # ================= phase A =================
        with ExitStack() as ctxA:
            pa = ctxA.enter_context(tc.tile_pool(name="pa", bufs=3, space="PSUM"))
            sp = ctxA.enter_context(tc.tile_pool(name="sp", bufs=1, space="PSUM"))
            act = ctxA.enter_context(tc.tile_pool(name="actA", bufs=1))
            scr = ctxA.enter_context(tc.tile_pool(name="scrA", bufs=2))

            # Full f32 on the score path (the top-16 must match the
            # reference exactly). All stages emit per-512-col halves so the
            # f=0 chain pipelines with f=1 across engines.
            hs = [act.tile([128, NL], F32, name=f"hh{m}") for m in range(2)]
            phs = [pa.tile([128, NL], F32, tag="mm", name=f"ph{m}")
                   for m in range(2)]
            pl3 = pa.tile([128, NL], F32, tag="mm")
            pse4 = sp.tile([4, NL], F32, tag="se")
            l3r = act.tile([NTAIL, NL], F32, name="l3r")
            e3r = act.tile([NTAIL, NL], F32, name="e3r")
            l4r = act.tile([4, NL], F32, name="l4r")
            for f in range(2):
                sl = slice(f * 512, (f + 1) * 512)
                for m in range(2):
                    nc.tensor.matmul(phs[m][:, sl],
                                     c64(O_W1 + m * 128, 128, dt=F32),
                                     c64(O_ZT + f * 512, 512, dt=F32),
                                     start=True, stop=True)
                    nc.scalar.activation(hs[m][:, sl], phs[m][:, sl], AF.Relu,
                                         bias=ca(A_B1 + m, 1, dt=F32))
                for kk in range(2):
                    nc.tensor.matmul(pl3[0:NTAIL, sl],
                                     ca(A_W2T + kk * NTAIL, NTAIL, dt=F32),
                                     hs[kk][:, sl],
                                     start=(kk == 0), stop=(kk == 1))
                nc.vector.tensor_copy(l3r[:, sl], pl3[0:NTAIL, sl])
                nc.scalar.activation(e3r[:, sl], pl3[0:NTAIL, sl], AF.Exp,
                                     bias=c64(O_B2T, 1, dt=F32))
                nc.tensor.matmul(pse4[:, sl], c64(O_G4, 4, dt=F32),
                                 e3r[:, sl], start=True, stop=True)
                nc.scalar.activation(l4r[:, sl], pse4[:, sl], AF.Ln)

            # scores scaled by 2^14 (folded into oht/coef on host), packed
            # as int32: clamp0(16384*s + 2^19) << 10 | local_id, top-8
            psts = [pa.tile([128, NL], F32, tag="mm", name=f"pst{bt}")
                    for bt in range(2)]
            pks = [scr.tile([128, NL], I32, tag=f"pk{bt}", name=f"pk{bt}")
                   for bt in range(2)]
            for f in range(2):
                sl = slice(f * 512, (f + 1) * 512)
                for bt in range(2):
                    nc.tensor.matmul(psts[bt][:, sl],
                                     c64(O_OHT + bt * 128, 128, dt=F32),
                                     l3r[:, sl], start=True, stop=False)
                    nc.tensor.matmul(psts[bt][:, sl],
                                     ca(A_COEF, 128, p=4, dt=F32),
                                     l4r[:, sl], start=False, stop=True)
                    t32 = scr.tile([128, 512], I32, tag="t32")
                    nc.vector.tensor_scalar(t32[:], psts[bt][:, sl],
                                            524288.0, 0.0,
                                            op0=ALU.add, op1=ALU.max)
                    _stt_int(nc.vector, pks[bt][:, sl], t32[:], 10,
                             lid[:, sl], ALU.logical_shift_left,
                             ALU.bitwise_or)
            for bt in range(2):
                x_sb = act.tile([128, 8], F32, name=f"x{bt}")
                nc.vector.max(x_sb[:], pks[bt][:].bitcast(F32))
                nc.sync.dma_start(xin[bt * 128:(bt + 1) * 128, :], x_sb[:])

        nc.gpsimd.collective_compute(
            "AllToAll", ALU.bypass, replica_groups=[list(range(P))],
            ins=[xin[:].opt()], outs=[xout[:].opt()],
        )

        # ================= phase B =================
        with ExitStack() as ctxB:
            pb = ctxB.enter_context(tc.tile_pool(name="pb", bufs=4, space="PSUM"))
            spb = ctxB.enter_context(tc.tile_pool(name="spb", bufs=1, space="PSUM"))
            ptr = ctxB.enter_context(tc.tile_pool(name="ptr", bufs=1, space="PSUM"))
            pnd_p = ctxB.enter_context(tc.tile_pool(name="pnd", bufs=1, space="PSUM"))
            act = ctxB.enter_context(tc.tile_pool(name="actB", bufs=1))

            # 64 packed candidates per local batch row
            cands = act.tile([BL, P * 8], F32, name="cands")
            nc.sync.dma_start(cands[:],
                              xout[:].rearrange("(s p) f -> p s f", s=P))
            wv = act.tile([BL, 16], F32, name="wv")
            nc.vector.max(wv[:, 0:8], cands[:])
            cm = act.tile([BL, P * 8], F32, name="cm")
            nc.vector.match_replace(cm[:], wv[:, 0:8], cands[:], 0.0)
            nc.vector.max(wv[:, 8:16], cm[:])

            # winner global ids: shard from slot position (pos//8)*1024,
            # local id from the packed low 10 bits
            ids32 = act.tile([32, 32], I32, name="ids32")
            nc.vector.memset(ids32[:], 0)
            pu = act.tile([BL, 16], mybir.dt.uint16, name="pu")
            nc.vector.max_index(pu[:, 0:8], wv[:, 0:8], cands[:])
            nc.vector.max_index(pu[:, 8:16], wv[:, 8:16], cands[:])
            posI = act.tile([BL, 16], I32, name="posI")
            nc.vector.tensor_copy(posI[:], pu[:])
            m1 = act.tile([BL, 16], I32, name="m1")
            _ts_int(nc.vector, m1[:], posI[:], 7, ALU.logical_shift_left)
            m2 = act.tile([BL, 16], I32, name="m2")
            _ts_int(nc.vector, m2[:], m1[:], 0x1C00, ALU.bitwise_and)
            _stt_int(nc.vector, ids32[:, 0:16], wv[:].bitcast(I32), 1023,
                     m2[:], ALU.bitwise_and, ALU.bitwise_or)
            idT = act.tile([32, 32], I32, name="idT")
            nc.vector.transpose(idT[:], ids32[:])
            idxall = act.tile([128, 32], I16, name="idxall")
            nc.vector.tensor_copy(idxall[0:16, :], idT[0:16, :])
            for g in [1, 2, 4]:
                nc.sync.dma_start(idxall[16 * g:16 * 2 * g, :],
                                  idxall[0:16 * g, :])
            nc.sync.dma_start(dbgi[:], idxall[0:16, :])

            # gather winner z rows from DRAM: ztop[p, i, :] = z[idx[i*128+p]]
            ztop = act.tile([128, 4, Lz], F32, name="ztop")
            nc.gpsimd.dma_gather(ztop[:], zr[:], idxall[:], num_idxs=512,
                                 num_idxs_reg=512, elem_size=Lz)
            # transpose to [64, 512] via PE
            pzt = ptr.tile([Lz, 512], F32, tag="tr")
            for i in range(4):
                nc.tensor.transpose(pzt[:, i * 128:(i + 1) * 128],
                                    ztop[:, i, :].bitcast(F32),
                                    cb(B_IDENT, 128, dt=F32))
            ztr = act.tile([Lz, 512], F32R, name="ztr")
            nc.scalar.copy(ztr[:], pzt[:])

            # h2 = relu(W1.T @ ztop + b1)
            h2s = []
            for m in range(2):
                ph2 = pb.tile([128, 512], F32, tag="mmb")
                nc.tensor.matmul(ph2[:], c64(O_W1 + m * 128, 128), ztr[:],
                                 start=True, stop=True)
                hh2 = act.tile([128, 512], F32R, name=f"hh2{m}")
                nc.scalar.activation(hh2[:], ph2[:], AF.Relu,
                                     bias=ca(A_B1 + m, 1, dt=F32))
                h2s.append(hh2)

            # full logits per t-tile -> exp -> per-feature sums
            pse2 = spb.tile([32, 512], F32, tag="seb")
            for t in range(4):
                pl2 = pb.tile([128, 512], F32, tag="mmb")
                for kk in range(2):
                    nc.tensor.matmul(pl2[:], cb(B_W2 + (kk * 4 + t) * 128, 128),
                                     h2s[kk][:], start=(kk == 0), stop=(kk == 1))
                e2r = act.tile([128, 512], F32R, name=f"e2r{t}")
                nc.scalar.activation(e2r[:], pl2[:], AF.Exp,
                                     bias=cb(B_B2E + t, 1, dt=F32))
                nc.tensor.matmul(pse2[:], cb(B_GSEL + t * 32, 32), e2r[:],
                                 start=(t == 0), stop=(t == 3))
            lgr = act.tile([32, 512], F32R, name="lgr")
            nc.scalar.activation(lgr[:], pse2[:], AF.Ln)

            # num (rows 0-31) and den (rows 32-63) in one PSUM tile
            pnd = pnd_p.tile([2 * BL, 512], F32, tag="nd")
            for kk in range(2):
                nc.tensor.matmul(pnd[:], cb(B_WOHND + kk * 64, 64), h2s[kk][:],
                                 start=(kk == 0), stop=False)
            nc.tensor.matmul(pnd[:], cb(B_COEFND, 64, p=32), lgr[:],
                             start=False, stop=True)
            ndf = act.tile([2 * BL, 512], F32, name="ndf")
            nc.scalar.copy(ndf[:], pnd[:])
            nc.sync.dma_start(dscr[:], ndf[:])

            # diagonal extraction: numd[b, j] = ndf[b, b*16+j] (+ den half)
            numd = act.tile([BL, 32], F32, name="numd")
            dnum = bass.AP(tensor=dscr[:].tensor, offset=0,
                           ap=[[512 + 16, BL], [1, 16]])
            nc.sync.dma_start(numd[:, 0:16], dnum)
            dden = bass.AP(tensor=dscr[:].tensor, offset=BL * 512,
                           ap=[[512 + 16, BL], [1, 16]])
            nc.sync.dma_start(numd[:, 16:32], dden)
            # fold oh.b2 constants (zero for this model, kept for generality)
            nc.vector.tensor_scalar_add(numd[:, 0:16], numd[:, 0:16],
                                        cb(B_CBND, 1, p=BL, dt=F32))
            nc.vector.tensor_scalar_add(numd[:, 16:32], numd[:, 16:32],
                                        cb(B_CBND + 1, 1, p=BL, dt=F32))

            # logsumexp(num) - logsumexp(den)
            ng = act.tile([BL, 2], F32, name="ng")
            nc.vector.tensor_reduce(ng[:, 0:1], numd[:, 0:16], axis=AX.X,
                                    op=ALU.max, negate=True)
            nc.vector.tensor_reduce(ng[:, 1:2], numd[:, 16:32], axis=AX.X,
                                    op=ALU.max, negate=True)
            s2 = act.tile([BL, 2], F32, name="s2")
            en = act.tile([BL, 16], F32, name="en")
            nc.scalar.activation(en[:], numd[:, 0:16], AF.Exp,
                                 bias=ng[:, 0:1], accum_out=s2[:, 0:1])
            ed = act.tile([BL, 16], F32, name="ed")
            nc.scalar.activation(ed[:], numd[:, 16:32], AF.Exp,
                                 bias=ng[:, 1:2], accum_out=s2[:, 1:2])
            lg = act.tile([BL, 2], F32, name="lg")
            nc.scalar.activation(lg[:], s2[:], AF.Ln)
            t1 = act.tile([BL, 1], F32, name="t1")
            nc.vector.tensor_sub(t1[:], lg[:, 0:1], lg[:, 1:2])
            t2 = act.tile([BL, 1], F32, name="t2")
            nc.vector.tensor_sub(t2[:], ng[:, 1:2], ng[:, 0:1])
            t3 = act.tile([BL, 1], F32, name="t3")
            nc.vector.tensor_add(t3[:], t1[:], t2[:])
            nc.sync.dma_start(outp[:], t3[:, 0])

    nc.compile()
    return nc


def _host_prep(x, z, W1, b1, W2, b2):
    oh = np.zeros((B, DC), np.float32)
    oh[np.arange(B)[:, None], np.arange(D)[None, :] * C + x] = 1.0
    oh_obs = oh.copy()
    oh_obs[:, DC - 4 * C:] = 0.0
    woh = oh @ W2.T          # (256, 256)
    wohd = oh_obs @ W2.T
    cbn = oh @ b2            # (256,)
    cbd = oh_obs @ b2

    k64c = np.zeros((Lz, C64), np.float32)
    k64c[:, O_W1:O_W1 + H] = W1
    # tail one-hot transposed, scaled by 2^14 for score packing
    k64c[:, O_OHT:O_OHT + B] = 16384.0 * oh[:, DC - NTAIL:].T
    g4 = np.zeros((Lz, 4), np.float32)
    g4[np.arange(Lz), np.arange(Lz) // C] = 1.0
    k64c[:, O_G4:O_G4 + 4] = g4
    k64c[:, O_B2T] = b2[DC - NTAIL:]

    kac = np.zeros((128, C128A), np.float32)
    for kk in range(2):
        kac[:, A_W2T + kk * NTAIL:A_W2T + (kk + 1) * NTAIL] = \
            W2[kk * 128:(kk + 1) * 128, DC - NTAIL:]
    kac[0:4, A_COEF:A_COEF + 128] = -16384.0
    kac[:, A_B1:A_B1 + 2] = b1.reshape(2, 128).T

    kbc = np.zeros((128, C128B), np.float32)
    for kk in range(2):
        for t in range(4):
            kbc[:, B_W2 + (kk * 4 + t) * 128:B_W2 + (kk * 4 + t + 1) * 128] = \
                W2[kk * 128:(kk + 1) * 128, t * 128:(t + 1) * 128]
    p_idx = np.arange(128)
    for t in range(4):
        kbc[p_idx, B_GSEL + t * 32 + t * 8 + p_idx // C] = 1.0
    kbc[0:32, B_COEFND:B_COEFND + 32] = -1.0
    kbc[0:28, B_COEFND + 32:B_COEFND + 64] = -1.0
    kbc[:, B_B2E:B_B2E + 4] = b2.reshape(4, 128).T
    kbc[:, B_IDENT:B_IDENT + 128] = np.eye(128, dtype=np.float32)
    kbc[:, B_MASK:B_MASK + 16] = np.full(
        (128, 16), np.int32(0x1C00)).view(np.float32)

    zf = np.ascontiguousarray(z, np.float32)
    in_maps = []
    for c in range(P):
        kc64 = k64c.copy()
        kc64[:, O_ZT:O_ZT + NL] = z[c * NL:(c + 1) * NL, :].T
        kca = kac.copy()
        kcb = kbc.copy()
        bsl = slice(c * BL, (c + 1) * BL)
        for kk in range(2):
            kcb[:, B_WOHND + kk * 64:B_WOHND + kk * 64 + BL] = \
                woh[bsl, kk * 128:(kk + 1) * 128].T
            kcb[:, B_WOHND + kk * 64 + BL:B_WOHND + (kk + 1) * 64] = \
                wohd[bsl, kk * 128:(kk + 1) * 128].T
        kcb[0:BL, B_CBND] = cbn[bsl]
        kcb[0:BL, B_CBND + 1] = cbd[bsl]
        in_maps.append(dict(pk64=kc64, pk128a=kca, pk128b=kcb, zr=zf))
    return in_maps


_NC_CACHE = {}


def kernel(x, log_w, z, k, W1, b1, W2, b2, _trace=False, _trace_kwargs=None):
    assert int(k) == K
    in_maps = _host_prep(np.asarray(x, np.int32), np.asarray(z, np.float32),
                         np.asarray(W1, np.float32), np.asarray(b1, np.float32),
                         np.asarray(W2, np.float32), np.asarray(b2, np.float32))
    if "nc" not in _NC_CACHE:
        _NC_CACHE["nc"] = _build_nc()
    nc = _NC_CACHE["nc"]
    res = run_bass_kernel_spmd(
        nc, in_maps, list(range(P)), trace=_trace, **(_trace_kwargs or {}))
    if _trace:
        _NC_CACHE["last_result"] = res
    return np.concatenate([np.asarray(res.results[c]["out"], np.float32)
                           for c in range(P)])
